# revision 35
# baseline (speedup 1.0000x reference)
"""IntraAttention Trainium2 kernel, 8-core SPMD, collective-free.

Reference computation (N=4096 rows, d=1024):
    Q = X @ Wq.T + bq ; K = X @ Wk.T + bk ; V = X @ Wv.T + bv
    alpha = softmax(Q @ K.T / sqrt(d), axis=1)
    V_ = alpha @ V
    x = concat([V_, Q], axis=1)              # [N, 2d]
    x1 = x @ Wl.T + bl                        # [N, d]
    h = x @ Wa.T + ba                         # [N, 2d]
    out = x1 * (h[:, :d] * sigmoid(h[:, d:]))

Key algebraic restructuring (removes all collectives): every core holds
the FULL X (fp8) plus its row shard X_c (fp16), and uses
    scores = Q K.T = X_c (Wq.T Wk) X.T + bq Wk X.T + (Q.bk) 1^T
           = G @ X.T + row-const           (softmax-invariant row-const)
      with G = X_c @ Wqk + bqk, Wqk = Wq.T Wk (host-precomputed), and
    alpha V = (alpha @ X) @ Wv.T + bv      (rows of alpha sum to 1)
so K and V are never materialized or gathered, and G does not depend on
Q. Per-core FLOPs are identical to the sharded-K/V formulation.

Precision: the attention path (G, scores, exp, U = exp@X, V_ = U@Wv.T,
and the V_-half of the x1/h projections) runs in fp8-e4m3 with
DoubleRow matmuls (2 fp8 MACs/cell/cycle, K=256 per instruction). The
Q path (Q projection, Q-half of x1/h) stays fp16: attention-path fp8
error is damped by softmax averaging (V_ is ~30x smaller than Q), while
Q feeds the output directly. Measured end-to-end rel err ~3e-3 vs the
2e-2 tolerance.

Schedule (PE order): G -> scores/exp/sums -> Q -> norm-broadcast -> U
-> V_ -> x1 -> h-a -> h-b+GLU. Q sits between the scores loop and the
broadcast matmul so the PE never waits on the softmax-sum reciprocal.
The x1/h projections accumulate the fp8-DR V_-half and the fp16 Q-half
into the same PSUM group, and h-b feeds the GLU sigmoid straight from
PSUM with ba folded into the activation bias.

DMA dispatches run in the issuing engine's instruction stream
(~0.7us each), so bulk tensors go out as single multi-descriptor
transfers: the sync queue carries the G-critical bytes + scores-loop
X.T tiles, the otherwise-idle GpSimd queue carries the X/Q/fp16-weight
streams, and the scalar queue keeps only its compute plus a few
consolidated fp8 weight loads.
"""

import numpy as np
import ml_dtypes

import concourse.bass as bass
import concourse.bacc as bacc
import concourse.tile as tile
import concourse.bass_utils as bass_utils
from concourse import mybir

P = 128            # partitions
D = 1024           # model dim
N = 4096           # rows
NCORES = 8
R = N // NCORES    # rows per core = 512
DC = D // P        # 128-wide d chunks = 8
C2 = D // (2 * P)  # 256-wide d chunks = 4
NT = N // P        # 128-key tiles = 32
NT2 = N // (2 * P)  # 256-key tiles = 16
TD = 2 * D         # 2048
TDC = TD // P      # 16

F32 = mybir.dt.float32
F16 = mybir.dt.float16
F8 = mybir.dt.float8e4
DR = mybir.MatmulPerfMode.DoubleRow
E4NP = ml_dtypes.float8_e4m3fn


def build_nc():
    nc = bacc.Bacc(
        "TRN2",
        target_bir_lowering=False,
        debug=False,
        num_devices=NCORES,
    )

    # ---- per-core I/O ----
    xtc8 = nc.dram_tensor("xtc8", [D, R], F8, kind="ExternalInput")     # X_c.T
    xt16 = nc.dram_tensor("xt16", [D, R], F16, kind="ExternalInput")    # X_c.T
    xt8 = nc.dram_tensor("xt8", [D, N], F8, kind="ExternalInput")       # X.T full
    x8 = nc.dram_tensor("x8", [N, D], F8, kind="ExternalInput")         # X full
    wqk8 = nc.dram_tensor("wqk8", [D, D], F8, kind="ExternalInput")     # Wq.T@Wk
    wqt = nc.dram_tensor("wqt", [D, D], F16, kind="ExternalInput")      # Wq.T
    wvt8 = nc.dram_tensor("wvt8", [D, D], F8, kind="ExternalInput")     # Wv.T
    wlv8 = nc.dram_tensor("wlv8", [D, D], F8, kind="ExternalInput")     # Wl.T[:d]
    wlq16 = nc.dram_tensor("wlq16", [D, D], F16, kind="ExternalInput")  # Wl.T[d:]
    wav8 = nc.dram_tensor("wav8", [D, TD], F8, kind="ExternalInput")    # Wa.T[:d]
    waq16 = nc.dram_tensor("waq16", [D, TD], F16, kind="ExternalInput")  # Wa.T[d:]
    bqk = nc.dram_tensor("bqk", [P, DC], F32, kind="ExternalInput")     # bq@Wk
    bq = nc.dram_tensor("bq", [P, DC], F32, kind="ExternalInput")
    bv = nc.dram_tensor("bv", [P, DC], F32, kind="ExternalInput")
    bl = nc.dram_tensor("bl", [P, DC], F32, kind="ExternalInput")
    ba = nc.dram_tensor("ba", [P, TDC], F32, kind="ExternalInput")
    out = nc.dram_tensor("out", [D, R], F16, kind="ExternalOutput")     # out_c.T

    with tile.TileContext(nc) as tc:
        with (
            tc.tile_pool(name="cpool", bufs=1) as cpool,
            tc.tile_pool(name="pspool", bufs=8, space="PSUM") as pspool,
        ):
            # constants (scalar queue; tiny)
            bqk_t = cpool.tile([P, DC], F32, name="bqk_t")
            bq_t = cpool.tile([P, DC], F32, name="bq_t")
            bv_t = cpool.tile([P, DC], F32, name="bv_t")
            bl_t = cpool.tile([P, DC], F32, name="bl_t")
            ba_t = cpool.tile([P, TDC], F32, name="ba_t")
            nc.scalar.dma_start(bqk_t, bqk[:, :])
            nc.scalar.dma_start(bq_t, bq[:, :])
            nc.scalar.dma_start(bv_t, bv[:, :])
            nc.scalar.dma_start(bl_t, bl[:, :])
            nc.scalar.dma_start(ba_t, ba[:, :])
            # DoubleRow-legal all-ones stationary: [P, 2, 128] (pair step 128,
            # full 128-partition output; every output row holds the key-sum)
            ones8 = cpool.tile([P, 2, P], F8, name="ones8")
            nc.vector.memset(ones8, 1.0)
            ones_row = cpool.tile([1, P], F32, name="ones_row")
            nc.vector.memset(ones_row, 1.0)

            with (
                tc.tile_pool(name="qpool", bufs=1) as qpool,
                tc.tile_pool(name="e8pool", bufs=1) as e8pool,
                tc.tile_pool(name="fpool", bufs=1) as fpool,
            ):
                qt_t = [qpool.tile([P, R], F16, name=f"qt{m}") for m in range(DC)]
                g8 = [qpool.tile([P, 2, R], F8, name=f"g8_{c}") for c in range(C2)]
                e8 = [e8pool.tile([P, 2, R], F8, name=f"e8_{i}") for i in range(NT2)]
                u8 = [fpool.tile([P, 2, R], F8, name=f"u8_{c}") for c in range(C2)]
                v8 = [fpool.tile([P, 2, R], F8, name=f"v8_{c}") for c in range(C2)]
                x1_t = [fpool.tile([P, R], F16, name=f"x1_{m}") for m in range(DC)]
                a_t = [fpool.tile([P, R], F16, name=f"a{m}") for m in range(DC)]
                bc_t = fpool.tile([P, R], F32, name="bc_t")

                with (
                    tc.tile_pool(name="xpool", bufs=1) as xpool,
                    tc.tile_pool(name="skpool", bufs=1) as skpool,
                ):
                    # ============ G.T = (X_c @ Wqk + bqk).T (fp8 DR) ============
                    # sync queue: G-critical halves, then the S-loop X.T tiles
                    wqk_h = [xpool.tile([P, 2 * 2, D], F8, name=f"wqk_h{h}")
                             for h in range(2)]
                    xc8_h = [xpool.tile([P, 2 * 2, R], F8, name=f"xc8_h{h}")
                             for h in range(2)]
                    for h in range(2):
                        nc.sync.dma_start(
                            wqk_h[h],
                            wqk8[h * D // 2:(h + 1) * D // 2, :]
                            .rearrange("(c j p) e -> p (c j) e", j=2, p=P))
                        nc.sync.dma_start(
                            xc8_h[h],
                            xtc8[h * D // 2:(h + 1) * D // 2, :]
                            .rearrange("(c j p) r -> p (c j) r", j=2, p=P))
                    # S-loop stationary X.T tiles (4MB fp8), folded for
                    # DoubleRow: [p, (c j), k] with d = c*256 + j*128 + p
                    xk_t = [skpool.tile([P, 2 * C2, 4 * P], F8, name=f"xk{g}")
                            for g in range(NT // 4)]
                    for g in range(NT // 4):
                        nc.sync.dma_start(
                            xk_t[g],
                            xt8[:, g * 4 * P:(g + 1) * 4 * P]
                            .rearrange("(c j p) k -> p (c j) k", j=2, p=P))
                    # Q-phase loads on the gpsimd queue (needed ~70us in; the
                    # first Q matmul needs all k chunks: m-outer)
                    xt_a = xpool.tile([P, DC, R], F16, name="xt_a")
                    nc.gpsimd.dma_start(
                        xt_a, xt16.rearrange("(k p) r -> p k r", p=P))
                    wq_a = xpool.tile([P, DC, D], F16, name="wq_a")
                    nc.gpsimd.dma_start(
                        wq_a, wqt.rearrange("(k p) e -> p k e", p=P))

                    g_ps = [pspool.tile([P, R], F32, name=f"gps{m}", tag="ps")
                            for m in range(DC)]
                    for c in range(C2):
                        h, cc = divmod(c, 2)
                        for m in range(DC):
                            nc.tensor.matmul(
                                g_ps[m],
                                wqk_h[h][:, 2 * cc:2 * cc + 2, m * P:(m + 1) * P],
                                xc8_h[h][:, 2 * cc:2 * cc + 2, :],
                                start=(c == 0), stop=(c == C2 - 1), perf_mode=DR)
                    for m in range(DC):
                        if m % 2 == 0:
                            nc.vector.tensor_scalar_add(
                                g8[m // 2][:, m % 2, :], g_ps[m], bqk_t[:, m:m + 1])
                        else:
                            nc.scalar.add(
                                g8[m // 2][:, m % 2, :], g_ps[m], bqk_t[:, m:m + 1])

                    # ============ scores.T -> exp (fp8 DR) + sums ============
                    sums_ps = pspool.tile([P, R], F32, name="sums_ps", tag="ps")

                    def sums_mm(i):
                        nc.tensor.matmul(
                            sums_ps, ones8, e8[i],
                            start=(i == 0), stop=(i == NT2 - 1),
                            perf_mode=DR, skip_group_check=True)

                    for t in range(NT):
                        g, u = divmod(t, 4)
                        sc_ps = pspool.tile([P, R], F32, name="sc_ps", tag="ps")
                        for c in range(C2):
                            nc.tensor.matmul(
                                sc_ps,
                                xk_t[g][:, 2 * c:2 * c + 2, u * P:(u + 1) * P],
                                g8[c],
                                start=(c == 0), stop=(c == C2 - 1), perf_mode=DR)
                        nc.scalar.activation(
                            e8[t // 2][:, t % 2, :], sc_ps,
                            mybir.ActivationFunctionType.Exp,
                            bias=0.0, scale=1.0 / 32.0)
                        if t % 2 == 1 and t >= 3:
                            sums_mm((t - 3) // 2)   # one behind: that pair is done
                    sums_mm(NT2 - 2)
                    sums_mm(NT2 - 1)
                    recip_t = cpool.tile([1, R], F32, name="recip_t")
                    nc.vector.reciprocal(recip_t, sums_ps[0:1, :])

                    # ============ Q = (X_c @ Wq.T + bq).T (fp16) ============
                    # Runs while the scalar engine drains the exp tail and the
                    # DVE computes the reciprocal, so the broadcast matmul
                    # below never stalls the PE. m-outer with a single rotating
                    # PSUM bank per block: evacuations trail the PE instead of
                    # piling up into an 8-bank release barrier before U.
                    for m in range(DC):
                        q_ps = pspool.tile([P, R], F32, name="q_ps", tag="ps")
                        for k in range(DC):
                            nc.tensor.matmul(
                                q_ps, wq_a[:, k, m * P:(m + 1) * P], xt_a[:, k, :],
                                start=(k == 0), stop=(k == DC - 1))
                        if m % 2 == 0:
                            nc.vector.tensor_scalar_add(qt_t[m], q_ps, bq_t[:, m:m + 1])
                        else:
                            nc.scalar.add(qt_t[m], q_ps, bq_t[:, m:m + 1])

                    # broadcast 1/sums to all partitions
                    bc_ps = pspool.tile([P, R], F32, name="bc_ps", tag="ps")
                    nc.tensor.matmul(bc_ps, ones_row, recip_t, start=True, stop=True)
                    nc.vector.tensor_copy(bc_t, bc_ps)

                with (
                    tc.tile_pool(name="xvpool", bufs=1) as xvpool,
                    tc.tile_pool(name="lwpool", bufs=3) as lwpool,
                    tc.tile_pool(name="fwpool", bufs=3) as fwpool,
                ):
                    # U-loop X rows: 4 groups of 1MB on the gpsimd queue;
                    # late-phase weights: fp8 consolidated on scalar, fp16
                    # consolidated on gpsimd.
                    xv_a = [xvpool.tile([P, 2 * C2, D], F8, name=f"xv{gr}")
                            for gr in range(NT2 // 4)]
                    for gr in range(NT2 // 4):
                        nc.gpsimd.dma_start(
                            xv_a[gr],
                            x8[gr * 8 * P:(gr + 1) * 8 * P, :]
                            .rearrange("(t j p) e -> p (t j) e", j=2, p=P))
                    wv_a = lwpool.tile([P, 2 * C2, D], F8, name="wv_a", tag="lw")
                    nc.scalar.dma_start(
                        wv_a, wvt8.rearrange("(c j p) e -> p (c j) e", j=2, p=P))
                    wlv_a = lwpool.tile([P, 2 * C2, D], F8, name="wlv_a", tag="lw")
                    nc.scalar.dma_start(
                        wlv_a, wlv8.rearrange("(c j p) e -> p (c j) e", j=2, p=P))
                    wl_a = fwpool.tile([P, DC, D], F16, name="wl_a", tag="fw")
                    nc.gpsimd.dma_start(
                        wl_a, wlq16.rearrange("(k p) e -> p k e", p=P))
                    wav_a = lwpool.tile([P, 2 * C2, D], F8, name="wav_a", tag="lw")
                    nc.scalar.dma_start(
                        wav_a,
                        wav8[:, 0:D].rearrange("(c j p) e -> p (c j) e", j=2, p=P))
                    wa_a = fwpool.tile([P, DC, D], F16, name="wa_a", tag="fw")
                    nc.gpsimd.dma_start(
                        wa_a,
                        waq16[:, 0:D].rearrange("(k p) e -> p k e", p=P))
                    wag_a = lwpool.tile([P, 2 * C2, D], F8, name="wag_a", tag="lw")
                    nc.scalar.dma_start(
                        wag_a,
                        wav8[:, D:TD].rearrange("(c j p) e -> p (c j) e", j=2, p=P))
                    wg_a = fwpool.tile([P, DC, D], F16, name="wg_a", tag="fw")
                    nc.gpsimd.dma_start(
                        wg_a,
                        waq16[:, D:TD].rearrange("(k p) e -> p k e", p=P))

                    # ============ U.T = (exp @ X).T (fp8 DR), normalize ========
                    vt_ps = [pspool.tile([P, R], F32, name=f"vtps{m}", tag="ps")
                             for m in range(DC)]
                    for t in range(NT2):
                        gr, u = divmod(t, 4)
                        for m in range(DC):
                            nc.tensor.matmul(
                                vt_ps[m],
                                xv_a[gr][:, 2 * u:2 * u + 2, m * P:(m + 1) * P],
                                e8[t],
                                start=(t == 0), stop=(t == NT2 - 1),
                                perf_mode=DR, skip_group_check=True)
                    for m in range(DC):
                        nc.vector.tensor_mul(u8[m // 2][:, m % 2, :], vt_ps[m], bc_t)

                    # ============ V_.T = (U @ Wv.T + bv).T (fp8 DR) ============
                    v_ps = [pspool.tile([P, R], F32, name=f"vps{m}", tag="ps")
                            for m in range(DC)]
                    for c in range(C2):
                        for m in range(DC):
                            nc.tensor.matmul(
                                v_ps[m], wv_a[:, 2 * c:2 * c + 2, m * P:(m + 1) * P],
                                u8[c],
                                start=(c == 0), stop=(c == C2 - 1), perf_mode=DR)
                    for m in range(DC):
                        if m % 2 == 0:
                            nc.vector.tensor_scalar_add(
                                v8[m // 2][:, m % 2, :], v_ps[m], bv_t[:, m:m + 1])
                        else:
                            nc.scalar.add(
                                v8[m // 2][:, m % 2, :], v_ps[m], bv_t[:, m:m + 1])

                    # ========= x1 = [V_, Q] @ Wl.T + bl  (DR + fp16) =========
                    x1_ps = [pspool.tile([P, R], F32, name=f"x1ps{m}", tag="ps")
                             for m in range(DC)]
                    for c in range(C2):
                        for m in range(DC):
                            nc.tensor.matmul(
                                x1_ps[m],
                                wlv_a[:, 2 * c:2 * c + 2, m * P:(m + 1) * P], v8[c],
                                start=(c == 0), stop=False, perf_mode=DR)
                    for k in range(DC):
                        for m in range(DC):
                            nc.tensor.matmul(
                                x1_ps[m], wl_a[:, k, m * P:(m + 1) * P], qt_t[k],
                                start=False, stop=(k == DC - 1))
                    for m in range(DC):
                        if m % 2 == 0:
                            nc.vector.tensor_scalar_add(
                                x1_t[m], x1_ps[m], bl_t[:, m:m + 1])
                        else:
                            nc.scalar.add(x1_t[m], x1_ps[m], bl_t[:, m:m + 1])

                    # ========= h g0 = a-part of [V_, Q] @ Wa.T + ba =========
                    h_ps = [pspool.tile([P, R], F32, name=f"hps{m}", tag="ps")
                            for m in range(DC)]
                    for c in range(C2):
                        for m in range(DC):
                            nc.tensor.matmul(
                                h_ps[m],
                                wav_a[:, 2 * c:2 * c + 2, m * P:(m + 1) * P], v8[c],
                                start=(c == 0), stop=False, perf_mode=DR)
                    for k in range(DC):
                        for m in range(DC):
                            nc.tensor.matmul(
                                h_ps[m], wa_a[:, k, m * P:(m + 1) * P], qt_t[k],
                                start=False, stop=(k == DC - 1))
                    for m in range(DC):
                        if m % 2 == 0:
                            nc.vector.tensor_scalar_add(
                                a_t[m], h_ps[m], ba_t[:, m:m + 1])
                        else:
                            nc.scalar.add(a_t[m], h_ps[m], ba_t[:, m:m + 1])

                    # ===== h g1 (b-part) m-outer, sigmoid from PSUM, GLU =====
                    with tc.tile_pool(name="gpool", bufs=4) as gpool:
                        for m in range(DC):
                            hg1 = pspool.tile([P, R], F32, name="hg1", tag="ps")
                            for c in range(C2):
                                nc.tensor.matmul(
                                    hg1,
                                    wag_a[:, 2 * c:2 * c + 2, m * P:(m + 1) * P],
                                    v8[c],
                                    start=(c == 0), stop=False, perf_mode=DR)
                            for k in range(DC):
                                nc.tensor.matmul(
                                    hg1, wg_a[:, k, m * P:(m + 1) * P], qt_t[k],
                                    start=False, stop=(k == DC - 1))
                            sig = gpool.tile([P, R], F32, name="sig", tag="g")
                            nc.scalar.activation(
                                sig, hg1, mybir.ActivationFunctionType.Sigmoid,
                                bias=ba_t[:, DC + m:DC + m + 1], scale=1.0)
                            nc.gpsimd.tensor_mul(a_t[m], a_t[m], sig)
                            og = gpool.tile([P, R], F16, name="og", tag="g")
                            nc.vector.tensor_mul(og, x1_t[m], a_t[m])
                            eng = nc.scalar if m % 2 == 0 else nc.sync
                            eng.dma_start(out[m * P:(m + 1) * P, :], og)

    nc.compile()
    return nc


_NC = None


def _get_nc():
    global _NC
    if _NC is None:
        _NC = build_nc()
    return _NC


def make_in_maps(input_features, Wq, bq, Wk, bk, Wv, bv, Wl, bl, Wa, ba):
    f = np.ascontiguousarray
    x = np.asarray(input_features, dtype=np.float32)
    xt = x.T
    xt16_full = f(xt.astype(np.float16))                 # [D, N]
    xt8_full = f(xt.astype(E4NP))                        # [D, N]
    x8_full = f(x.astype(E4NP))                          # [N, D]
    wq = np.asarray(Wq, np.float32)
    wk = np.asarray(Wk, np.float32)
    wqk8 = f((wq.T @ wk).astype(E4NP))                   # [D, D]
    bqk = np.asarray(bq, np.float32) @ wk                # [D]
    wqt = f(wq.T.astype(np.float16))
    wvt8 = f(np.asarray(Wv, np.float32).T.astype(E4NP))
    wlt = np.asarray(Wl, np.float32).T                   # [2D, D]
    wlv8 = f(wlt[:D].astype(E4NP))
    wlq16 = f(wlt[D:].astype(np.float16))
    wat = np.asarray(Wa, np.float32).T                   # [2D, 2D]
    wav8 = f(wat[:D].astype(E4NP))
    waq16 = f(wat[D:].astype(np.float16))
    bqk_r = f(bqk.reshape(DC, P).T)                      # [P, DC]
    bq_r = f(np.asarray(bq, np.float32).reshape(DC, P).T)
    bv_r = f(np.asarray(bv, np.float32).reshape(DC, P).T)
    bl_r = f(np.asarray(bl, np.float32).reshape(DC, P).T)
    ba_r = f(np.asarray(ba, np.float32).reshape(TDC, P).T)     # [P, TDC]
    in_maps = []
    for c in range(NCORES):
        in_maps.append({
            "xtc8": f(xt8_full[:, c * R:(c + 1) * R]),
            "xt16": f(xt16_full[:, c * R:(c + 1) * R]),
            "xt8": xt8_full, "x8": x8_full,
            "wqk8": wqk8, "wqt": wqt, "wvt8": wvt8,
            "wlv8": wlv8, "wlq16": wlq16, "wav8": wav8, "waq16": waq16,
            "bqk": bqk_r, "bq": bq_r, "bv": bv_r, "bl": bl_r, "ba": ba_r,
        })
    return in_maps


def run(in_maps, trace=False):
    nc = _get_nc()
    return bass_utils.run_bass_kernel_spmd(
        nc, in_maps, core_ids=list(range(NCORES)), trace=trace)


def kernel(input_features, Wq, bq, Wk, bk, Wv, bv, Wl, bl, Wa, ba):
    in_maps = make_in_maps(input_features, Wq, bq, Wk, bk, Wv, bv, Wl, bl, Wa, ba)
    res = run(in_maps)
    out = np.empty((N, D), dtype=np.float32)
    for c in range(NCORES):
        out[c * R:(c + 1) * R, :] = res.results[c]["out"].T.astype(np.float32)
    return out


# revision 40
# speedup vs baseline: 1.0102x; 1.0102x over previous
"""IntraAttention Trainium2 kernel, 8-core SPMD, collective-free.

Reference computation (N=4096 rows, d=1024):
    Q = X @ Wq.T + bq ; K = X @ Wk.T + bk ; V = X @ Wv.T + bv
    alpha = softmax(Q @ K.T / sqrt(d), axis=1)
    V_ = alpha @ V
    x = concat([V_, Q], axis=1)              # [N, 2d]
    x1 = x @ Wl.T + bl                        # [N, d]
    h = x @ Wa.T + ba                         # [N, 2d]
    out = x1 * (h[:, :d] * sigmoid(h[:, d:]))

Key algebraic restructuring (removes all collectives): every core holds
the FULL X (fp8) plus its row shard X_c (fp16), and uses
    scores = Q K.T = X_c (Wq.T Wk) X.T + bq Wk X.T + (Q.bk) 1^T
           = G @ X.T + row-const           (softmax-invariant row-const)
      with G = X_c @ Wqk + bqk, Wqk = Wq.T Wk (host-precomputed), and
    alpha V = (alpha @ X) @ Wv.T + bv      (rows of alpha sum to 1)
so K and V are never materialized or gathered, and G does not depend on
Q. Per-core FLOPs are identical to the sharded-K/V formulation.

Precision: the attention path (G, scores, exp, U = exp@X, V_ = U@Wv.T,
and the V_-half of the x1/h projections) runs in fp8-e4m3 with
DoubleRow matmuls (2 fp8 MACs/cell/cycle, K=256 per instruction). The
Q path (Q projection, Q-half of x1/h) stays fp16: attention-path fp8
error is damped by softmax averaging (V_ is ~30x smaller than Q), while
Q feeds the output directly. Measured end-to-end rel err ~3e-3 vs the
2e-2 tolerance.

Schedule (PE order): G -> scores/exp/sums -> Q -> norm-broadcast -> U
-> V_ -> x1 -> h-a -> h-b+GLU. Q sits between the scores loop and the
broadcast matmul so the PE never waits on the softmax-sum reciprocal.
The x1/h projections accumulate the fp8-DR V_-half and the fp16 Q-half
into the same PSUM group, and h-b feeds the GLU sigmoid straight from
PSUM with ba folded into the activation bias.

DMA dispatches run in the issuing engine's instruction stream
(~0.7us each), so bulk tensors go out as single multi-descriptor
transfers: the sync queue carries the G-critical bytes + scores-loop
X.T tiles, the otherwise-idle GpSimd queue carries the X/Q/fp16-weight
streams, and the scalar queue keeps only its compute plus a few
consolidated fp8 weight loads.
"""

import numpy as np
import ml_dtypes

import concourse.bass as bass
import concourse.bacc as bacc
import concourse.tile as tile
import concourse.bass_utils as bass_utils
from concourse import mybir

P = 128            # partitions
D = 1024           # model dim
N = 4096           # rows
NCORES = 8
R = N // NCORES    # rows per core = 512
DC = D // P        # 128-wide d chunks = 8
C2 = D // (2 * P)  # 256-wide d chunks = 4
NT = N // P        # 128-key tiles = 32
NT2 = N // (2 * P)  # 256-key tiles = 16
TD = 2 * D         # 2048
TDC = TD // P      # 16

F32 = mybir.dt.float32
F16 = mybir.dt.float16
F8 = mybir.dt.float8e4
DR = mybir.MatmulPerfMode.DoubleRow
E4NP = ml_dtypes.float8_e4m3fn


def build_nc():
    nc = bacc.Bacc(
        "TRN2",
        target_bir_lowering=False,
        debug=False,
        num_devices=NCORES,
    )

    # ---- per-core I/O ----
    xtc8 = nc.dram_tensor("xtc8", [D, R], F8, kind="ExternalInput")     # X_c.T
    xt16 = nc.dram_tensor("xt16", [D, R], F16, kind="ExternalInput")    # X_c.T
    xt8 = nc.dram_tensor("xt8", [D, N], F8, kind="ExternalInput")       # X.T full
    x8 = nc.dram_tensor("x8", [N, D], F8, kind="ExternalInput")         # X full
    wqk8 = nc.dram_tensor("wqk8", [D, D], F8, kind="ExternalInput")     # Wq.T@Wk
    wqt = nc.dram_tensor("wqt", [D, D], F16, kind="ExternalInput")      # Wq.T
    wvt8 = nc.dram_tensor("wvt8", [D, D], F8, kind="ExternalInput")     # Wv.T
    wlv8 = nc.dram_tensor("wlv8", [D, D], F8, kind="ExternalInput")     # Wl.T[:d]
    wlq16 = nc.dram_tensor("wlq16", [D, D], F16, kind="ExternalInput")  # Wl.T[d:]
    wav8 = nc.dram_tensor("wav8", [D, TD], F8, kind="ExternalInput")    # Wa.T[:d]
    waq16 = nc.dram_tensor("waq16", [D, TD], F16, kind="ExternalInput")  # Wa.T[d:]
    bqk = nc.dram_tensor("bqk", [P, DC], F32, kind="ExternalInput")     # bq@Wk
    bq = nc.dram_tensor("bq", [P, DC], F32, kind="ExternalInput")
    bv = nc.dram_tensor("bv", [P, DC], F32, kind="ExternalInput")
    bl = nc.dram_tensor("bl", [P, DC], F32, kind="ExternalInput")
    ba = nc.dram_tensor("ba", [P, TDC], F32, kind="ExternalInput")
    out = nc.dram_tensor("out", [D, R], F16, kind="ExternalOutput")     # out_c.T

    with tile.TileContext(nc) as tc:
        with (
            tc.tile_pool(name="cpool", bufs=1) as cpool,
            tc.tile_pool(name="pspool", bufs=8, space="PSUM") as pspool,
        ):
            # constants (scalar queue; tiny)
            bqk_t = cpool.tile([P, DC], F32, name="bqk_t")
            bq_t = cpool.tile([P, DC], F32, name="bq_t")
            bv_t = cpool.tile([P, DC], F32, name="bv_t")
            bl_t = cpool.tile([P, DC], F32, name="bl_t")
            ba_t = cpool.tile([P, TDC], F32, name="ba_t")
            nc.scalar.dma_start(bqk_t, bqk[:, :])
            nc.scalar.dma_start(bq_t, bq[:, :])
            nc.scalar.dma_start(bv_t, bv[:, :])
            nc.scalar.dma_start(bl_t, bl[:, :])
            nc.scalar.dma_start(ba_t, ba[:, :])
            # DoubleRow-legal all-ones stationary: [P, 2, 128] (pair step 128,
            # full 128-partition output; every output row holds the key-sum)
            ones8 = cpool.tile([P, 2, P], F8, name="ones8")
            nc.vector.memset(ones8, 1.0)
            ones_row = cpool.tile([1, P], F32, name="ones_row")
            nc.vector.memset(ones_row, 1.0)

            with (
                tc.tile_pool(name="qpool", bufs=1) as qpool,
                tc.tile_pool(name="e8pool", bufs=1) as e8pool,
                tc.tile_pool(name="fpool", bufs=1) as fpool,
            ):
                qt_t = [qpool.tile([P, R], F16, name=f"qt{m}") for m in range(DC)]
                g8 = [qpool.tile([P, 2, R], F8, name=f"g8_{c}") for c in range(C2)]
                e8 = [e8pool.tile([P, 2, R], F8, name=f"e8_{i}") for i in range(NT2)]
                u8 = [fpool.tile([P, 2, R], F8, name=f"u8_{c}") for c in range(C2)]
                v8 = [fpool.tile([P, 2, R], F8, name=f"v8_{c}") for c in range(C2)]
                x1_t = [fpool.tile([P, R], F16, name=f"x1_{m}") for m in range(DC)]
                a_t = [fpool.tile([P, R], F16, name=f"a{m}") for m in range(DC)]
                bc_t = fpool.tile([P, R], F32, name="bc_t")
                xv_a = [fpool.tile([P, 2 * C2, D], F8, name=f"xv{gr}")
                        for gr in range(NT2 // 4)]

                with (
                    tc.tile_pool(name="xpool", bufs=1) as xpool,
                    tc.tile_pool(name="skpool", bufs=1) as skpool,
                ):
                    # ============ G.T = (X_c @ Wqk + bqk).T (fp8 DR) ============
                    # Deadline-critical streams spread across all five engine
                    # DMA queues (each drains ~50GB/s): the tensor queue
                    # head-starts the G-critical bytes before its first
                    # matmul; sync and vector interleave the scores-loop X.T
                    # tiles and U-loop X rows; gpsimd takes the Q loads.
                    wqk_h = [xpool.tile([P, 2 * 2, D], F8, name=f"wqk_h{h}")
                             for h in range(2)]
                    xc8_h = [xpool.tile([P, 2 * 2, R], F8, name=f"xc8_h{h}")
                             for h in range(2)]
                    xk_t = [skpool.tile([P, 2 * C2, 4 * P], F8, name=f"xk{g}")
                            for g in range(NT // 4)]
                    xt_h = [xpool.tile([P, DC // 2, R], F16, name=f"xt_h{h}")
                            for h in range(2)]
                    wq_h = [xpool.tile([P, DC // 2, D], F16, name=f"wq_h{h}")
                            for h in range(2)]

                    def load_wqk(eng, h):
                        eng.dma_start(
                            wqk_h[h],
                            wqk8[h * D // 2:(h + 1) * D // 2, :]
                            .rearrange("(c j p) e -> p (c j) e", j=2, p=P))
                        eng.dma_start(
                            xc8_h[h],
                            xtc8[h * D // 2:(h + 1) * D // 2, :]
                            .rearrange("(c j p) r -> p (c j) r", j=2, p=P))

                    def load_xk(eng, g):
                        eng.dma_start(
                            xk_t[g],
                            xt8[:, g * 4 * P:(g + 1) * 4 * P]
                            .rearrange("(c j p) k -> p (c j) k", j=2, p=P))

                    def load_xv(eng, gr):
                        eng.dma_start(
                            xv_a[gr],
                            x8[gr * 8 * P:(gr + 1) * 8 * P, :]
                            .rearrange("(t j p) e -> p (t j) e", j=2, p=P))

                    def load_xtwq(eng, h):
                        eng.dma_start(
                            xt_h[h],
                            xt16[h * D // 2:(h + 1) * D // 2, :]
                            .rearrange("(k p) r -> p k r", p=P))
                        eng.dma_start(
                            wq_h[h],
                            wqt[h * D // 2:(h + 1) * D // 2, :]
                            .rearrange("(k p) e -> p k e", p=P))

                    # per-queue streams in deadline order; all dispatched
                    # up front (~0.7us of sequencer time each)
                    load_wqk(nc.sync, 0)
                    load_xk(nc.sync, 0)
                    load_xk(nc.sync, 3)
                    load_xk(nc.sync, 6)
                    load_xtwq(nc.sync, 0)
                    load_xv(nc.sync, 0)
                    load_xv(nc.sync, 3)
                    load_wqk(nc.scalar, 1)
                    load_xk(nc.scalar, 1)
                    load_xk(nc.scalar, 4)
                    load_xk(nc.scalar, 7)
                    load_xtwq(nc.scalar, 1)
                    load_xv(nc.scalar, 1)
                    load_xk(nc.gpsimd, 2)
                    load_xk(nc.gpsimd, 5)
                    load_xv(nc.gpsimd, 2)

                    g_ps = [pspool.tile([P, R], F32, name=f"gps{m}", tag="ps")
                            for m in range(DC)]
                    for c in range(C2):
                        h, cc = divmod(c, 2)
                        for m in range(DC):
                            nc.tensor.matmul(
                                g_ps[m],
                                wqk_h[h][:, 2 * cc:2 * cc + 2, m * P:(m + 1) * P],
                                xc8_h[h][:, 2 * cc:2 * cc + 2, :],
                                start=(c == 0), stop=(c == C2 - 1), perf_mode=DR)
                    for m in range(DC):
                        if m % 2 == 0:
                            nc.vector.tensor_scalar_add(
                                g8[m // 2][:, m % 2, :], g_ps[m], bqk_t[:, m:m + 1])
                        else:
                            nc.scalar.add(
                                g8[m // 2][:, m % 2, :], g_ps[m], bqk_t[:, m:m + 1])

                    # ============ scores.T -> exp (fp8 DR) + sums ============
                    sums_ps = pspool.tile([P, R], F32, name="sums_ps", tag="ps")

                    def sums_mm(i):
                        nc.tensor.matmul(
                            sums_ps, ones8, e8[i],
                            start=(i == 0), stop=(i == NT2 - 1),
                            perf_mode=DR, skip_group_check=True)

                    for t in range(NT):
                        g, u = divmod(t, 4)
                        sc_ps = pspool.tile([P, R], F32, name="sc_ps", tag="ps")
                        for c in range(C2):
                            nc.tensor.matmul(
                                sc_ps,
                                xk_t[g][:, 2 * c:2 * c + 2, u * P:(u + 1) * P],
                                g8[c],
                                start=(c == 0), stop=(c == C2 - 1), perf_mode=DR)
                        nc.scalar.activation(
                            e8[t // 2][:, t % 2, :], sc_ps,
                            mybir.ActivationFunctionType.Exp,
                            bias=0.0, scale=1.0 / 32.0)
                        if t % 2 == 1 and t >= 3:
                            sums_mm((t - 3) // 2)   # one behind: that pair is done
                    sums_mm(NT2 - 2)
                    sums_mm(NT2 - 1)
                    recip_t = cpool.tile([1, R], F32, name="recip_t")
                    nc.vector.reciprocal(recip_t, sums_ps[0:1, :])

                    # ============ Q = (X_c @ Wq.T + bq).T (fp16) ============
                    # Runs while the scalar engine drains the exp tail and the
                    # DVE computes the reciprocal, so the broadcast matmul
                    # below never stalls the PE. k-outer so the first matmuls
                    # need only the first half of the xt/wq streams.
                    q_ps = [pspool.tile([P, R], F32, name=f"qps{m}", tag="ps")
                            for m in range(DC)]
                    for k in range(DC):
                        h, kk = divmod(k, DC // 2)
                        for m in range(DC):
                            nc.tensor.matmul(
                                q_ps[m], wq_h[h][:, kk, m * P:(m + 1) * P],
                                xt_h[h][:, kk, :],
                                start=(k == 0), stop=(k == DC - 1))
                    for m in range(DC):
                        if m % 2 == 0:
                            nc.vector.tensor_scalar_add(
                                qt_t[m], q_ps[m], bq_t[:, m:m + 1])
                        else:
                            nc.scalar.add(qt_t[m], q_ps[m], bq_t[:, m:m + 1])

                    # broadcast 1/sums to all partitions
                    bc_ps = pspool.tile([P, R], F32, name="bc_ps", tag="ps")
                    nc.tensor.matmul(bc_ps, ones_row, recip_t, start=True, stop=True)
                    nc.vector.tensor_copy(bc_t, bc_ps)

                with (
                    tc.tile_pool(name="lwpool", bufs=3) as lwpool,
                    tc.tile_pool(name="fwpool", bufs=3) as fwpool,
                ):
                    # late-phase weights: fp8 consolidated on scalar, fp16
                    # consolidated on gpsimd
                    wv_a = lwpool.tile([P, 2 * C2, D], F8, name="wv_a", tag="lw")
                    nc.scalar.dma_start(
                        wv_a, wvt8.rearrange("(c j p) e -> p (c j) e", j=2, p=P))
                    wlv_a = lwpool.tile([P, 2 * C2, D], F8, name="wlv_a", tag="lw")
                    nc.scalar.dma_start(
                        wlv_a, wlv8.rearrange("(c j p) e -> p (c j) e", j=2, p=P))
                    wl_a = fwpool.tile([P, DC, D], F16, name="wl_a", tag="fw")
                    nc.gpsimd.dma_start(
                        wl_a, wlq16.rearrange("(k p) e -> p k e", p=P))
                    wav_a = lwpool.tile([P, 2 * C2, D], F8, name="wav_a", tag="lw")
                    nc.scalar.dma_start(
                        wav_a,
                        wav8[:, 0:D].rearrange("(c j p) e -> p (c j) e", j=2, p=P))
                    wa_a = fwpool.tile([P, DC, D], F16, name="wa_a", tag="fw")
                    nc.gpsimd.dma_start(
                        wa_a,
                        waq16[:, 0:D].rearrange("(k p) e -> p k e", p=P))
                    wag_a = lwpool.tile([P, 2 * C2, D], F8, name="wag_a", tag="lw")
                    nc.scalar.dma_start(
                        wag_a,
                        wav8[:, D:TD].rearrange("(c j p) e -> p (c j) e", j=2, p=P))
                    wg_a = fwpool.tile([P, DC, D], F16, name="wg_a", tag="fw")
                    nc.gpsimd.dma_start(
                        wg_a,
                        waq16[:, D:TD].rearrange("(k p) e -> p k e", p=P))

                    # ============ U.T = (exp @ X).T (fp8 DR), normalize ========
                    vt_ps = [pspool.tile([P, R], F32, name=f"vtps{m}", tag="ps")
                             for m in range(DC)]
                    for t in range(NT2):
                        gr, u = divmod(t, 4)
                        for m in range(DC):
                            nc.tensor.matmul(
                                vt_ps[m],
                                xv_a[gr][:, 2 * u:2 * u + 2, m * P:(m + 1) * P],
                                e8[t],
                                start=(t == 0), stop=(t == NT2 - 1),
                                perf_mode=DR, skip_group_check=True)
                    for m in range(DC):
                        nc.vector.tensor_mul(u8[m // 2][:, m % 2, :], vt_ps[m], bc_t)

                    # ============ V_.T = (U @ Wv.T + bv).T (fp8 DR) ============
                    v_ps = [pspool.tile([P, R], F32, name=f"vps{m}", tag="ps")
                            for m in range(DC)]
                    for c in range(C2):
                        for m in range(DC):
                            nc.tensor.matmul(
                                v_ps[m], wv_a[:, 2 * c:2 * c + 2, m * P:(m + 1) * P],
                                u8[c],
                                start=(c == 0), stop=(c == C2 - 1), perf_mode=DR)
                    for m in range(DC):
                        if m % 2 == 0:
                            nc.vector.tensor_scalar_add(
                                v8[m // 2][:, m % 2, :], v_ps[m], bv_t[:, m:m + 1])
                        else:
                            nc.scalar.add(
                                v8[m // 2][:, m % 2, :], v_ps[m], bv_t[:, m:m + 1])

                    # ========= x1 = [V_, Q] @ Wl.T + bl  (DR + fp16) =========
                    x1_ps = [pspool.tile([P, R], F32, name=f"x1ps{m}", tag="ps")
                             for m in range(DC)]
                    for c in range(C2):
                        for m in range(DC):
                            nc.tensor.matmul(
                                x1_ps[m],
                                wlv_a[:, 2 * c:2 * c + 2, m * P:(m + 1) * P], v8[c],
                                start=(c == 0), stop=False, perf_mode=DR)
                    for k in range(DC):
                        for m in range(DC):
                            nc.tensor.matmul(
                                x1_ps[m], wl_a[:, k, m * P:(m + 1) * P], qt_t[k],
                                start=False, stop=(k == DC - 1))
                    for m in range(DC):
                        if m % 2 == 0:
                            nc.vector.tensor_scalar_add(
                                x1_t[m], x1_ps[m], bl_t[:, m:m + 1])
                        else:
                            nc.scalar.add(x1_t[m], x1_ps[m], bl_t[:, m:m + 1])

                    # ========= h g0 = a-part of [V_, Q] @ Wa.T + ba =========
                    h_ps = [pspool.tile([P, R], F32, name=f"hps{m}", tag="ps")
                            for m in range(DC)]
                    for c in range(C2):
                        for m in range(DC):
                            nc.tensor.matmul(
                                h_ps[m],
                                wav_a[:, 2 * c:2 * c + 2, m * P:(m + 1) * P], v8[c],
                                start=(c == 0), stop=False, perf_mode=DR)
                    for k in range(DC):
                        for m in range(DC):
                            nc.tensor.matmul(
                                h_ps[m], wa_a[:, k, m * P:(m + 1) * P], qt_t[k],
                                start=False, stop=(k == DC - 1))
                    for m in range(DC):
                        if m % 2 == 0:
                            nc.vector.tensor_scalar_add(
                                a_t[m], h_ps[m], ba_t[:, m:m + 1])
                        else:
                            nc.scalar.add(a_t[m], h_ps[m], ba_t[:, m:m + 1])

                    # ===== h g1 (b-part) m-outer, sigmoid from PSUM, GLU =====
                    with tc.tile_pool(name="gpool", bufs=4) as gpool:
                        for m in range(DC):
                            hg1 = pspool.tile([P, R], F32, name="hg1", tag="ps")
                            for c in range(C2):
                                nc.tensor.matmul(
                                    hg1,
                                    wag_a[:, 2 * c:2 * c + 2, m * P:(m + 1) * P],
                                    v8[c],
                                    start=(c == 0), stop=False, perf_mode=DR)
                            for k in range(DC):
                                nc.tensor.matmul(
                                    hg1, wg_a[:, k, m * P:(m + 1) * P], qt_t[k],
                                    start=False, stop=(k == DC - 1))
                            sig = gpool.tile([P, R], F32, name="sig", tag="g")
                            nc.scalar.activation(
                                sig, hg1, mybir.ActivationFunctionType.Sigmoid,
                                bias=ba_t[:, DC + m:DC + m + 1], scale=1.0)
                            nc.gpsimd.tensor_mul(a_t[m], a_t[m], sig)
                            og = gpool.tile([P, R], F16, name="og", tag="g")
                            nc.vector.tensor_mul(og, x1_t[m], a_t[m])
                            eng = nc.scalar if m % 2 == 0 else nc.sync
                            eng.dma_start(out[m * P:(m + 1) * P, :], og)

    nc.compile()
    return nc


_NC = None


def _get_nc():
    global _NC
    if _NC is None:
        _NC = build_nc()
    return _NC


def make_in_maps(input_features, Wq, bq, Wk, bk, Wv, bv, Wl, bl, Wa, ba):
    f = np.ascontiguousarray
    x = np.asarray(input_features, dtype=np.float32)
    xt = x.T
    xt16_full = f(xt.astype(np.float16))                 # [D, N]
    xt8_full = f(xt.astype(E4NP))                        # [D, N]
    x8_full = f(x.astype(E4NP))                          # [N, D]
    wq = np.asarray(Wq, np.float32)
    wk = np.asarray(Wk, np.float32)
    wqk8 = f((wq.T @ wk).astype(E4NP))                   # [D, D]
    bqk = np.asarray(bq, np.float32) @ wk                # [D]
    wqt = f(wq.T.astype(np.float16))
    wvt8 = f(np.asarray(Wv, np.float32).T.astype(E4NP))
    wlt = np.asarray(Wl, np.float32).T                   # [2D, D]
    wlv8 = f(wlt[:D].astype(E4NP))
    wlq16 = f(wlt[D:].astype(np.float16))
    wat = np.asarray(Wa, np.float32).T                   # [2D, 2D]
    wav8 = f(wat[:D].astype(E4NP))
    waq16 = f(wat[D:].astype(np.float16))
    bqk_r = f(bqk.reshape(DC, P).T)                      # [P, DC]
    bq_r = f(np.asarray(bq, np.float32).reshape(DC, P).T)
    bv_r = f(np.asarray(bv, np.float32).reshape(DC, P).T)
    bl_r = f(np.asarray(bl, np.float32).reshape(DC, P).T)
    ba_r = f(np.asarray(ba, np.float32).reshape(TDC, P).T)     # [P, TDC]
    in_maps = []
    for c in range(NCORES):
        in_maps.append({
            "xtc8": f(xt8_full[:, c * R:(c + 1) * R]),
            "xt16": f(xt16_full[:, c * R:(c + 1) * R]),
            "xt8": xt8_full, "x8": x8_full,
            "wqk8": wqk8, "wqt": wqt, "wvt8": wvt8,
            "wlv8": wlv8, "wlq16": wlq16, "wav8": wav8, "waq16": waq16,
            "bqk": bqk_r, "bq": bq_r, "bv": bv_r, "bl": bl_r, "ba": ba_r,
        })
    return in_maps


def run(in_maps, trace=False):
    nc = _get_nc()
    return bass_utils.run_bass_kernel_spmd(
        nc, in_maps, core_ids=list(range(NCORES)), trace=trace)


def kernel(input_features, Wq, bq, Wk, bk, Wv, bv, Wl, bl, Wa, ba):
    in_maps = make_in_maps(input_features, Wq, bq, Wk, bk, Wv, bv, Wl, bl, Wa, ba)
    res = run(in_maps)
    out = np.empty((N, D), dtype=np.float32)
    for c in range(NCORES):
        out[c * R:(c + 1) * R, :] = res.results[c]["out"].T.astype(np.float32)
    return out


# revision 43
# speedup vs baseline: 1.0458x; 1.0353x over previous
"""IntraAttention Trainium2 kernel, 8-core SPMD, collective-free.

Reference computation (N=4096 rows, d=1024):
    Q = X @ Wq.T + bq ; K = X @ Wk.T + bk ; V = X @ Wv.T + bv
    alpha = softmax(Q @ K.T / sqrt(d), axis=1)
    V_ = alpha @ V
    x = concat([V_, Q], axis=1)              # [N, 2d]
    x1 = x @ Wl.T + bl                        # [N, d]
    h = x @ Wa.T + ba                         # [N, 2d]
    out = x1 * (h[:, :d] * sigmoid(h[:, d:]))

Key algebraic restructuring (removes all collectives): every core holds
the FULL X (fp8) plus its row shard X_c (fp16), and uses
    scores = Q K.T = X_c (Wq.T Wk) X.T + bq Wk X.T + (Q.bk) 1^T
           = G @ X.T + row-const           (softmax-invariant row-const)
      with G = X_c @ Wqk + bqk, Wqk = Wq.T Wk (host-precomputed), and
    alpha V = (alpha @ X) @ Wv.T + bv      (rows of alpha sum to 1)
so K and V are never materialized or gathered, and G does not depend on
Q. Per-core FLOPs are identical to the sharded-K/V formulation.

Precision: the attention path (G, scores, exp, U = exp@X, V_ = U@Wv.T,
and the V_-half of the x1/h projections) runs in fp8-e4m3 with
DoubleRow matmuls (2 fp8 MACs/cell/cycle, K=256 per instruction). The
Q path (Q projection, Q-half of x1/h) stays fp16: attention-path fp8
error is damped by softmax averaging (V_ is ~30x smaller than Q), while
Q feeds the output directly. Measured end-to-end rel err ~3e-3 vs the
2e-2 tolerance.

Schedule (PE order): G -> scores/exp/sums -> Q -> norm-broadcast -> U
-> V_ -> x1 -> h-a -> h-b+GLU. Q sits between the scores loop and the
broadcast matmul so the PE never waits on the softmax-sum reciprocal.
The x1/h projections accumulate the fp8-DR V_-half and the fp16 Q-half
into the same PSUM group, and h-b feeds the GLU sigmoid straight from
PSUM with ba folded into the activation bias.

DMA dispatches run in the issuing engine's instruction stream
(~0.7us each), so bulk tensors go out as single multi-descriptor
transfers: the sync queue carries the G-critical bytes + scores-loop
X.T tiles, the otherwise-idle GpSimd queue carries the X/Q/fp16-weight
streams, and the scalar queue keeps only its compute plus a few
consolidated fp8 weight loads.
"""

import numpy as np
import ml_dtypes

import concourse.bass as bass
import concourse.bacc as bacc
import concourse.tile as tile
import concourse.bass_utils as bass_utils
from concourse import mybir

P = 128            # partitions
D = 1024           # model dim
N = 4096           # rows
NCORES = 8
R = N // NCORES    # rows per core = 512
DC = D // P        # 128-wide d chunks = 8
C2 = D // (2 * P)  # 256-wide d chunks = 4
NT = N // P        # 128-key tiles = 32
NT2 = N // (2 * P)  # 256-key tiles = 16
TD = 2 * D         # 2048
TDC = TD // P      # 16

F32 = mybir.dt.float32
F16 = mybir.dt.float16
F8 = mybir.dt.float8e4
DR = mybir.MatmulPerfMode.DoubleRow
E4NP = ml_dtypes.float8_e4m3fn


def build_nc():
    nc = bacc.Bacc(
        "TRN2",
        target_bir_lowering=False,
        debug=False,
        num_devices=NCORES,
    )

    # ---- per-core I/O ----
    xtc8 = nc.dram_tensor("xtc8", [D, R], F8, kind="ExternalInput")     # X_c.T
    xt16 = nc.dram_tensor("xt16", [D, R], F16, kind="ExternalInput")    # X_c.T
    xt8 = nc.dram_tensor("xt8", [D, N], F8, kind="ExternalInput")       # X.T full
    x8 = nc.dram_tensor("x8", [N, D], F8, kind="ExternalInput")         # X full
    wqk8 = nc.dram_tensor("wqk8", [D, D], F8, kind="ExternalInput")     # Wq.T@Wk
    wqt = nc.dram_tensor("wqt", [D, D], F16, kind="ExternalInput")      # Wq.T
    wvt8 = nc.dram_tensor("wvt8", [D, D], F8, kind="ExternalInput")     # Wv.T
    wlv8 = nc.dram_tensor("wlv8", [D, D], F8, kind="ExternalInput")     # Wl.T[:d]
    wlq16 = nc.dram_tensor("wlq16", [D, D], F16, kind="ExternalInput")  # Wl.T[d:]
    wav8 = nc.dram_tensor("wav8", [D, TD], F8, kind="ExternalInput")    # Wa.T[:d]
    waq16 = nc.dram_tensor("waq16", [D, TD], F16, kind="ExternalInput")  # Wa.T[d:]
    bqk = nc.dram_tensor("bqk", [P, DC], F32, kind="ExternalInput")     # bq@Wk
    bq = nc.dram_tensor("bq", [P, DC], F32, kind="ExternalInput")
    bv = nc.dram_tensor("bv", [P, DC], F32, kind="ExternalInput")
    bl = nc.dram_tensor("bl", [P, DC], F32, kind="ExternalInput")
    ba = nc.dram_tensor("ba", [P, TDC], F32, kind="ExternalInput")
    out = nc.dram_tensor("out", [D, R], F16, kind="ExternalOutput")     # out_c.T

    with tile.TileContext(nc) as tc:
        with (
            tc.tile_pool(name="cpool", bufs=1) as cpool,
            tc.tile_pool(name="pspool", bufs=8, space="PSUM") as pspool,
        ):
            # constants (scalar queue; tiny)
            bqk_t = cpool.tile([P, DC], F32, name="bqk_t")
            bq_t = cpool.tile([P, DC], F32, name="bq_t")
            bv_t = cpool.tile([P, DC], F32, name="bv_t")
            bl_t = cpool.tile([P, DC], F32, name="bl_t")
            ba_t = cpool.tile([P, TDC], F32, name="ba_t")
            nc.scalar.dma_start(bqk_t, bqk[:, :])
            nc.scalar.dma_start(bq_t, bq[:, :])
            nc.scalar.dma_start(bv_t, bv[:, :])
            nc.scalar.dma_start(bl_t, bl[:, :])
            nc.scalar.dma_start(ba_t, ba[:, :])
            # DoubleRow-legal all-ones stationary: [P, 2, 128] (pair step 128,
            # full 128-partition output; every output row holds the key-sum)
            ones8 = cpool.tile([P, 2, P], F8, name="ones8")
            nc.vector.memset(ones8, 1.0)
            ones_row = cpool.tile([1, P], F32, name="ones_row")
            nc.vector.memset(ones_row, 1.0)

            with (
                tc.tile_pool(name="qpool", bufs=1) as qpool,
                tc.tile_pool(name="e8pool", bufs=1) as e8pool,
                tc.tile_pool(name="fpool", bufs=1) as fpool,
            ):
                qt_t = [qpool.tile([P, R], F16, name=f"qt{m}") for m in range(DC)]
                g8 = [qpool.tile([P, 2, R], F8, name=f"g8_{c}") for c in range(C2)]
                e8 = [e8pool.tile([P, 2, R], F8, name=f"e8_{i}") for i in range(NT2)]
                u8 = [fpool.tile([P, 2, R], F8, name=f"u8_{c}") for c in range(C2)]
                v8 = [fpool.tile([P, 2, R], F8, name=f"v8_{c}") for c in range(C2)]
                x1_t = [fpool.tile([P, R], F16, name=f"x1_{m}") for m in range(DC)]
                a_t = [fpool.tile([P, R], F16, name=f"a{m}") for m in range(DC)]
                bc_t = fpool.tile([P, R], F32, name="bc_t")
                xv_a = [fpool.tile([P, 2 * C2, D], F8, name=f"xv{gr}")
                        for gr in range(NT2 // 4)]

                with (
                    tc.tile_pool(name="xpool", bufs=1) as xpool,
                    tc.tile_pool(name="skpool", bufs=1) as skpool,
                ):
                    # ============ G.T = (X_c @ Wqk + bqk).T (fp8 DR) ============
                    # Deadline-critical streams spread across all five engine
                    # DMA queues (each drains ~50GB/s): the tensor queue
                    # head-starts the G-critical bytes before its first
                    # matmul; sync and vector interleave the scores-loop X.T
                    # tiles and U-loop X rows; gpsimd takes the Q loads.
                    wqk_q = [xpool.tile([P, 2, D], F8, name=f"wqk_q{c}")
                             for c in range(C2)]
                    xc8_q = [xpool.tile([P, 2, R], F8, name=f"xc8_q{c}")
                             for c in range(C2)]
                    xk_t = [skpool.tile([P, 2 * C2, 4 * P], F8, name=f"xk{g}")
                            for g in range(NT // 4)]
                    xt_h = [xpool.tile([P, DC // 2, R], F16, name=f"xt_h{h}")
                            for h in range(2)]
                    wq_h = [xpool.tile([P, DC // 2, D], F16, name=f"wq_h{h}")
                            for h in range(2)]

                    def load_wqk(eng, c):
                        eng.dma_start(
                            wqk_q[c],
                            wqk8[c * 2 * P:(c + 1) * 2 * P, :]
                            .rearrange("(j p) e -> p j e", p=P))

                    def load_xc8(eng, c):
                        eng.dma_start(
                            xc8_q[c],
                            xtc8[c * 2 * P:(c + 1) * 2 * P, :]
                            .rearrange("(j p) r -> p j r", p=P))

                    def load_xk(eng, g):
                        eng.dma_start(
                            xk_t[g],
                            xt8[:, g * 4 * P:(g + 1) * 4 * P]
                            .rearrange("(c j p) k -> p (c j) k", j=2, p=P))

                    def load_xv(eng, gr):
                        eng.dma_start(
                            xv_a[gr],
                            x8[gr * 8 * P:(gr + 1) * 8 * P, :]
                            .rearrange("(t j p) e -> p (t j) e", j=2, p=P))

                    def load_xtwq(eng, h):
                        eng.dma_start(
                            xt_h[h],
                            xt16[h * D // 2:(h + 1) * D // 2, :]
                            .rearrange("(k p) r -> p k r", p=P))
                        eng.dma_start(
                            wq_h[h],
                            wqt[h * D // 2:(h + 1) * D // 2, :]
                            .rearrange("(k p) e -> p k e", p=P))

                    # per-queue streams in deadline order (each queue drains
                    # one transfer at a time at ~45GB/s, so the G-critical
                    # quarters are spread across all three queues in
                    # consumption order)
                    load_wqk(nc.sync, 0)
                    load_xc8(nc.scalar, 0)
                    load_wqk(nc.gpsimd, 1)
                    load_xc8(nc.sync, 1)
                    load_wqk(nc.sync, 2)
                    load_xc8(nc.gpsimd, 2)
                    load_wqk(nc.scalar, 3)
                    load_xc8(nc.gpsimd, 3)
                    load_xk(nc.sync, 0)
                    load_xk(nc.scalar, 1)
                    load_xk(nc.gpsimd, 2)
                    load_xk(nc.sync, 3)
                    load_xk(nc.scalar, 4)
                    load_xk(nc.gpsimd, 5)
                    load_xk(nc.sync, 6)
                    load_xk(nc.gpsimd, 7)
                    load_xtwq(nc.sync, 0)
                    load_xtwq(nc.scalar, 1)
                    load_xv(nc.sync, 0)
                    load_xv(nc.scalar, 1)
                    load_xv(nc.gpsimd, 2)
                    load_xv(nc.sync, 3)

                    g_ps = [pspool.tile([P, R], F32, name=f"gps{m}", tag="ps")
                            for m in range(DC)]
                    for c in range(C2):
                        for m in range(DC):
                            nc.tensor.matmul(
                                g_ps[m], wqk_q[c][:, :, m * P:(m + 1) * P], xc8_q[c],
                                start=(c == 0), stop=(c == C2 - 1), perf_mode=DR)
                    for m in range(DC):
                        if m % 2 == 0:
                            nc.vector.tensor_scalar_add(
                                g8[m // 2][:, m % 2, :], g_ps[m], bqk_t[:, m:m + 1])
                        else:
                            nc.scalar.add(
                                g8[m // 2][:, m % 2, :], g_ps[m], bqk_t[:, m:m + 1])

                    # ============ scores.T -> exp (fp8 DR) + sums ============
                    sums_ps = pspool.tile([P, R], F32, name="sums_ps", tag="ps")

                    def sums_mm(i):
                        nc.tensor.matmul(
                            sums_ps, ones8, e8[i],
                            start=(i == 0), stop=(i == NT2 - 1),
                            perf_mode=DR, skip_group_check=True)

                    for t in range(NT):
                        g, u = divmod(t, 4)
                        sc_ps = pspool.tile([P, R], F32, name="sc_ps", tag="ps")
                        for c in range(C2):
                            nc.tensor.matmul(
                                sc_ps,
                                xk_t[g][:, 2 * c:2 * c + 2, u * P:(u + 1) * P],
                                g8[c],
                                start=(c == 0), stop=(c == C2 - 1), perf_mode=DR)
                        nc.scalar.activation(
                            e8[t // 2][:, t % 2, :], sc_ps,
                            mybir.ActivationFunctionType.Exp,
                            bias=0.0, scale=1.0 / 32.0)
                        if t % 2 == 1 and t >= 3:
                            sums_mm((t - 3) // 2)   # one behind: that pair is done
                    sums_mm(NT2 - 2)
                    sums_mm(NT2 - 1)
                    recip_t = cpool.tile([1, R], F32, name="recip_t")
                    nc.vector.reciprocal(recip_t, sums_ps[0:1, :])

                    # ============ Q = (X_c @ Wq.T + bq).T (fp16) ============
                    # Runs while the scalar engine drains the exp tail and the
                    # DVE computes the reciprocal, so the broadcast matmul
                    # below never stalls the PE. k-outer so the first matmuls
                    # need only the first half of the xt/wq streams.
                    q_ps = [pspool.tile([P, R], F32, name=f"qps{m}", tag="ps")
                            for m in range(DC)]
                    for k in range(DC):
                        h, kk = divmod(k, DC // 2)
                        for m in range(DC):
                            nc.tensor.matmul(
                                q_ps[m], wq_h[h][:, kk, m * P:(m + 1) * P],
                                xt_h[h][:, kk, :],
                                start=(k == 0), stop=(k == DC - 1))
                    for m in range(DC):
                        if m % 2 == 0:
                            nc.vector.tensor_scalar_add(
                                qt_t[m], q_ps[m], bq_t[:, m:m + 1])
                        else:
                            nc.scalar.add(qt_t[m], q_ps[m], bq_t[:, m:m + 1])

                    # broadcast 1/sums to all partitions
                    bc_ps = pspool.tile([P, R], F32, name="bc_ps", tag="ps")
                    nc.tensor.matmul(bc_ps, ones_row, recip_t, start=True, stop=True)
                    nc.vector.tensor_copy(bc_t, bc_ps)

                with (
                    tc.tile_pool(name="lwpool", bufs=3) as lwpool,
                    tc.tile_pool(name="fwpool", bufs=3) as fwpool,
                ):
                    # late-phase weights: fp8 consolidated on scalar, fp16
                    # consolidated on gpsimd
                    wv_a = lwpool.tile([P, 2 * C2, D], F8, name="wv_a", tag="lw")
                    nc.scalar.dma_start(
                        wv_a, wvt8.rearrange("(c j p) e -> p (c j) e", j=2, p=P))
                    wlv_a = lwpool.tile([P, 2 * C2, D], F8, name="wlv_a", tag="lw")
                    nc.scalar.dma_start(
                        wlv_a, wlv8.rearrange("(c j p) e -> p (c j) e", j=2, p=P))
                    wl_a = fwpool.tile([P, DC, D], F16, name="wl_a", tag="fw")
                    nc.gpsimd.dma_start(
                        wl_a, wlq16.rearrange("(k p) e -> p k e", p=P))
                    wav_a = lwpool.tile([P, 2 * C2, D], F8, name="wav_a", tag="lw")
                    nc.scalar.dma_start(
                        wav_a,
                        wav8[:, 0:D].rearrange("(c j p) e -> p (c j) e", j=2, p=P))
                    wa_a = fwpool.tile([P, DC, D], F16, name="wa_a", tag="fw")
                    nc.gpsimd.dma_start(
                        wa_a,
                        waq16[:, 0:D].rearrange("(k p) e -> p k e", p=P))
                    wag_a = lwpool.tile([P, 2 * C2, D], F8, name="wag_a", tag="lw")
                    nc.scalar.dma_start(
                        wag_a,
                        wav8[:, D:TD].rearrange("(c j p) e -> p (c j) e", j=2, p=P))
                    wg_a = fwpool.tile([P, DC, D], F16, name="wg_a", tag="fw")
                    nc.gpsimd.dma_start(
                        wg_a,
                        waq16[:, D:TD].rearrange("(k p) e -> p k e", p=P))

                    # ============ U.T = (exp @ X).T (fp8 DR), normalize ========
                    vt_ps = [pspool.tile([P, R], F32, name=f"vtps{m}", tag="ps")
                             for m in range(DC)]
                    for t in range(NT2):
                        gr, u = divmod(t, 4)
                        for m in range(DC):
                            nc.tensor.matmul(
                                vt_ps[m],
                                xv_a[gr][:, 2 * u:2 * u + 2, m * P:(m + 1) * P],
                                e8[t],
                                start=(t == 0), stop=(t == NT2 - 1),
                                perf_mode=DR, skip_group_check=True)
                    for m in range(DC):
                        nc.vector.tensor_mul(u8[m // 2][:, m % 2, :], vt_ps[m], bc_t)

                    # ============ V_.T = (U @ Wv.T + bv).T (fp8 DR) ============
                    v_ps = [pspool.tile([P, R], F32, name=f"vps{m}", tag="ps")
                            for m in range(DC)]
                    for c in range(C2):
                        for m in range(DC):
                            nc.tensor.matmul(
                                v_ps[m], wv_a[:, 2 * c:2 * c + 2, m * P:(m + 1) * P],
                                u8[c],
                                start=(c == 0), stop=(c == C2 - 1), perf_mode=DR)
                    for m in range(DC):
                        if m % 2 == 0:
                            nc.vector.tensor_scalar_add(
                                v8[m // 2][:, m % 2, :], v_ps[m], bv_t[:, m:m + 1])
                        else:
                            nc.scalar.add(
                                v8[m // 2][:, m % 2, :], v_ps[m], bv_t[:, m:m + 1])

                    # ========= x1 = [V_, Q] @ Wl.T + bl  (DR + fp16) =========
                    x1_ps = [pspool.tile([P, R], F32, name=f"x1ps{m}", tag="ps")
                             for m in range(DC)]
                    for c in range(C2):
                        for m in range(DC):
                            nc.tensor.matmul(
                                x1_ps[m],
                                wlv_a[:, 2 * c:2 * c + 2, m * P:(m + 1) * P], v8[c],
                                start=(c == 0), stop=False, perf_mode=DR)
                    for k in range(DC):
                        for m in range(DC):
                            nc.tensor.matmul(
                                x1_ps[m], wl_a[:, k, m * P:(m + 1) * P], qt_t[k],
                                start=False, stop=(k == DC - 1))
                    for m in range(DC):
                        if m % 2 == 0:
                            nc.vector.tensor_scalar_add(
                                x1_t[m], x1_ps[m], bl_t[:, m:m + 1])
                        else:
                            nc.scalar.add(x1_t[m], x1_ps[m], bl_t[:, m:m + 1])

                    # ========= h g0 = a-part of [V_, Q] @ Wa.T + ba =========
                    h_ps = [pspool.tile([P, R], F32, name=f"hps{m}", tag="ps")
                            for m in range(DC)]
                    for c in range(C2):
                        for m in range(DC):
                            nc.tensor.matmul(
                                h_ps[m],
                                wav_a[:, 2 * c:2 * c + 2, m * P:(m + 1) * P], v8[c],
                                start=(c == 0), stop=False, perf_mode=DR)
                    for k in range(DC):
                        for m in range(DC):
                            nc.tensor.matmul(
                                h_ps[m], wa_a[:, k, m * P:(m + 1) * P], qt_t[k],
                                start=False, stop=(k == DC - 1))
                    for m in range(DC):
                        if m % 2 == 0:
                            nc.vector.tensor_scalar_add(
                                a_t[m], h_ps[m], ba_t[:, m:m + 1])
                        else:
                            nc.scalar.add(a_t[m], h_ps[m], ba_t[:, m:m + 1])

                    # ===== h g1 (b-part) m-outer, sigmoid from PSUM, GLU =====
                    with tc.tile_pool(name="gpool", bufs=4) as gpool:
                        for m in range(DC):
                            hg1 = pspool.tile([P, R], F32, name="hg1", tag="ps")
                            for c in range(C2):
                                nc.tensor.matmul(
                                    hg1,
                                    wag_a[:, 2 * c:2 * c + 2, m * P:(m + 1) * P],
                                    v8[c],
                                    start=(c == 0), stop=False, perf_mode=DR)
                            for k in range(DC):
                                nc.tensor.matmul(
                                    hg1, wg_a[:, k, m * P:(m + 1) * P], qt_t[k],
                                    start=False, stop=(k == DC - 1))
                            sig = gpool.tile([P, R], F32, name="sig", tag="g")
                            nc.scalar.activation(
                                sig, hg1, mybir.ActivationFunctionType.Sigmoid,
                                bias=ba_t[:, DC + m:DC + m + 1], scale=1.0)
                            nc.gpsimd.tensor_mul(a_t[m], a_t[m], sig)
                            og = gpool.tile([P, R], F16, name="og", tag="g")
                            nc.vector.tensor_mul(og, x1_t[m], a_t[m])
                            eng = nc.scalar if m % 2 == 0 else nc.sync
                            eng.dma_start(out[m * P:(m + 1) * P, :], og)

    nc.compile()
    return nc


_NC = None


def _get_nc():
    global _NC
    if _NC is None:
        _NC = build_nc()
    return _NC


def make_in_maps(input_features, Wq, bq, Wk, bk, Wv, bv, Wl, bl, Wa, ba):
    f = np.ascontiguousarray
    x = np.asarray(input_features, dtype=np.float32)
    xt = x.T
    xt16_full = f(xt.astype(np.float16))                 # [D, N]
    xt8_full = f(xt.astype(E4NP))                        # [D, N]
    x8_full = f(x.astype(E4NP))                          # [N, D]
    wq = np.asarray(Wq, np.float32)
    wk = np.asarray(Wk, np.float32)
    wqk8 = f((wq.T @ wk).astype(E4NP))                   # [D, D]
    bqk = np.asarray(bq, np.float32) @ wk                # [D]
    wqt = f(wq.T.astype(np.float16))
    wvt8 = f(np.asarray(Wv, np.float32).T.astype(E4NP))
    wlt = np.asarray(Wl, np.float32).T                   # [2D, D]
    wlv8 = f(wlt[:D].astype(E4NP))
    wlq16 = f(wlt[D:].astype(np.float16))
    wat = np.asarray(Wa, np.float32).T                   # [2D, 2D]
    wav8 = f(wat[:D].astype(E4NP))
    waq16 = f(wat[D:].astype(np.float16))
    bqk_r = f(bqk.reshape(DC, P).T)                      # [P, DC]
    bq_r = f(np.asarray(bq, np.float32).reshape(DC, P).T)
    bv_r = f(np.asarray(bv, np.float32).reshape(DC, P).T)
    bl_r = f(np.asarray(bl, np.float32).reshape(DC, P).T)
    ba_r = f(np.asarray(ba, np.float32).reshape(TDC, P).T)     # [P, TDC]
    in_maps = []
    for c in range(NCORES):
        in_maps.append({
            "xtc8": f(xt8_full[:, c * R:(c + 1) * R]),
            "xt16": f(xt16_full[:, c * R:(c + 1) * R]),
            "xt8": xt8_full, "x8": x8_full,
            "wqk8": wqk8, "wqt": wqt, "wvt8": wvt8,
            "wlv8": wlv8, "wlq16": wlq16, "wav8": wav8, "waq16": waq16,
            "bqk": bqk_r, "bq": bq_r, "bv": bv_r, "bl": bl_r, "ba": ba_r,
        })
    return in_maps


def run(in_maps, trace=False):
    nc = _get_nc()
    return bass_utils.run_bass_kernel_spmd(
        nc, in_maps, core_ids=list(range(NCORES)), trace=trace)


def kernel(input_features, Wq, bq, Wk, bk, Wv, bv, Wl, bl, Wa, ba):
    in_maps = make_in_maps(input_features, Wq, bq, Wk, bk, Wv, bv, Wl, bl, Wa, ba)
    res = run(in_maps)
    out = np.empty((N, D), dtype=np.float32)
    for c in range(NCORES):
        out[c * R:(c + 1) * R, :] = res.results[c]["out"].T.astype(np.float32)
    return out
